# revision 34
# baseline (speedup 1.0000x reference)
"""Distributed MoE (top-2 routing, capacity 320) on 8 Trainium2 NeuronCores.

Sharding (matches the expert-parallel hint):
  - x is data-parallel sharded along B: core b owns batch row b (2048 tokens).
  - W1/b1/W2/b2 are sharded along the expert dim: core e owns expert e.
  - The router (Wg, bg) is replicated; each core routes its own tokens.
  - Dispatch: each core scatters its tokens into a [E, CAP, C] buffer and an
    AllToAll moves expert-e slabs to core e, which then holds [B, CAP, C]
    tokens for its expert. After the expert FFN a second AllToAll returns
    [E, CAP, C] outputs to each data-parallel core, which combines them with
    the gate probabilities.

Everything (router matmul, softmax, top-2, capacity positions via a prefix
scan, scatter/gather via indirect DMA, the two AllToAlls, and the expert FFN)
runs on-device; the host only slices/reassembles numpy arrays.

Key implementation points:
  - The dispatch buffer uses a chunk-major layout (row = j*G + e*CH + pos%CH,
    j = pos//CH) so each AllToAll is split into NG=5 chunked collectives whose
    transfers overlap the expert FFN groups (group g consumes chunk g).
  - Expert weights are passed host-pre-tiled as [out_chunk, 128, K*128] so one
    DMA per 128-wide output chunk loads all contraction tiles with contiguous
    16KB partition lines (the naive per-tile layout saturated the in-order
    sync sequencer and starved the PE).
  - FFN output DMAs ride the ACT HWDGE ring so they never block the sync ring
    that streams weights.
  - The token-position cumsum is a chained `tensor_tensor_scan`, fused per
    token tile so routing, index build, and dispatch scatter pipeline.
  - Matmuls default to fp16 (1 cycle/row vs 4 for fp32's two half-speed
    passes; measured rel. error 4.2e-4 vs the f32 reference, routing/top-k
    decisions are computed in exact f32 and match the reference bit-for-bit).
"""

import numpy as np

import concourse.mybir as mybir
import concourse.tile as tile
from concourse import bacc
from concourse.bass import IndirectOffsetOnAxis
from concourse.bass_utils import run_bass_kernel_spmd
from concourse.masks import make_identity

F32 = mybir.dt.float32
I32 = mybir.dt.int32
U32 = mybir.dt.uint32
AX = mybir.AxisListType
ALU = mybir.AluOpType
ACTF = mybir.ActivationFunctionType

P = 128


def build_moe_nc(T=2048, C=1024, E=8, CAP=320, DFF=4096, dt_mm1=F32, dt_mm2=F32, zero_disp=False):
    """Build the per-core (SPMD) Bass program. All 8 cores run this module."""
    assert T % P == 0 and C % P == 0 and DFF % P == 0
    NT = T // P         # token tiles per core
    KC = C // P         # C chunks (contraction for matmul1)
    KD = DFF // P       # DFF chunks (contraction for matmul2)
    ECAP = E * CAP      # rows in the dispatch buffer
    G = 512 if ECAP % 512 == 0 else ECAP   # FFN token-group size / A2A chunk rows
    assert ECAP % G == 0 and G % P == 0
    NG = ECAP // G      # FFN groups == A2A chunks
    NS = G // P         # 128-token subtiles per group
    CH = G // E         # capacity rows per (expert, chunk)
    SH = CH.bit_length() - 1
    assert (1 << SH) == CH, "chunk size must be a power of two"
    GSH = G.bit_length() - 1
    assert (1 << GSH) == G, "group size must be a power of two"
    assert CAP == NG * CH
    cores = list(range(E))

    nc = bacc.Bacc(None, target_bir_lowering=False, debug=False)

    # ---- I/O (per core) --------------------------------------------------
    x_ext = nc.dram_tensor("x", [T, C], F32, kind="ExternalInput")
    wg_ext = nc.dram_tensor("wgt", [P, KC, E], F32, kind="ExternalInput")   # Wg[C,E] -> [P, KC, E]
    bg_ext = nc.dram_tensor("bg", [1, E], F32, kind="ExternalInput")
    w1_ext = nc.dram_tensor("w1t", [KD, P, KC * P], dt_mm1, kind="ExternalInput")
    b1_ext = nc.dram_tensor("b1t", [P, KD], F32, kind="ExternalInput")
    w2_ext = nc.dram_tensor("w2t", [KC, P, KD * P], dt_mm2, kind="ExternalInput")
    b2_ext = nc.dram_tensor("b2t", [P, KC], F32, kind="ExternalInput")
    out_ext = nc.dram_tensor("out", [T, C], F32, kind="ExternalOutput")

    with tile.TileContext(nc) as tc:
        with (
            tc.tile_pool(name="const", bufs=1) as constp,
            tc.tile_pool(name="dram", bufs=1, space="DRAM") as dramp,
            tc.tile_pool(name="route", bufs=1) as routep,
        ):
            # ---- internal DRAM (collective + staging buffers) ----
            disp = dramp.tile([ECAP, C], F32)    # my tokens, per-expert slabs
            recv = dramp.tile([ECAP, C], F32)    # post-A2A: my expert, per-src slabs
            ysend = dramp.tile([ECAP, C], F32)   # expert outputs, per-src slabs
            recv2 = dramp.tile([ECAP, C], F32)   # post-A2A: my tokens' expert outputs

            # ---- constants ----
            ident = constp.tile([P, P], F32)
            make_identity(nc, ident)
            wg_sb = constp.tile([P, KC * E], F32)
            nc.sync.dma_start(wg_sb[:], wg_ext[:])
            bg_sb = constp.tile([1, E], F32)
            nc.sync.dma_start(bg_sb[:], bg_ext[:])
            ones1 = constp.tile([1, P], F32)
            nc.vector.memset(ones1[:], 1.0)
            ones8 = constp.tile([8, 1], F32)
            nc.vector.memset(ones8[:], 1.0)
            b1_sb = constp.tile([P, KD], F32)
            nc.sync.dma_start(b1_sb[:], b1_ext[:])
            b2_sb = constp.tile([P, KC], F32)
            nc.sync.dma_start(b2_sb[:], b2_ext[:])

            # ---- persistent routing tables (small; survive into combine) ----
            metas = [routep.tile([P, 8], F32, tag=f"meta{i}", name=f"meta{i}") for i in range(NT)]
            idxs = [routep.tile([P, 4], I32, tag=f"idx{i}", name=f"idx{i}") for i in range(NT)]

            # ================= Phase A: router + top-2 ====================
            with (
                tc.tile_pool(name="xa", bufs=1) as xap,
                tc.tile_pool(name="xtp", bufs=4) as xtp,
                tc.tile_pool(name="apsA", bufs=2, space="PSUM") as apsA,
                tc.tile_pool(name="apsB", bufs=2, space="PSUM") as apsB,
                tc.tile_pool(name="apsC", bufs=2, space="PSUM") as apsC,
                tc.tile_pool(name="apsD", bufs=1, space="PSUM") as apsD,
                tc.tile_pool(name="asb", bufs=4) as asb,
                tc.tile_pool(name="ascr", bufs=1) as ascr,
            ):
                # phase-A scratch (freed before the FFN needs the SBUF)
                SST = ascr.tile([8, T], F32)          # chained cumsum of expert one-hots
                if zero_disp:
                    # unfilled capacity slots never reach the output; zeroing
                    # only satisfies the simulator's NaN checker (emitted
                    # before the scatters, ordered via Tile WAW deps)
                    zt = asb.tile([P, C], F32, tag="zt", bufs=1)
                    nc.vector.memset(zt[:], 0.0)
                    for j in range(ECAP // P):
                        nc.gpsimd.dma_start(disp[j * P:(j + 1) * P, :], zt[:])
                x_tiles = []
                for i in range(NT):
                    x_t = xap.tile([P, C], F32, tag=f"x{i}", name=f"x{i}")
                    x_tiles.append(x_t)
                    nc.scalar.dma_start(x_t[:], x_ext[i * P:(i + 1) * P, :])
                    # transpose x tile -> xT (C on partitions)
                    xT = xtp.tile([P, C], F32, tag="xT")
                    PK = min(4, KC)
                    for h in range(KC // PK):
                        xt_ps = apsA.tile([P, PK * P], F32, tag="xt_ps")
                        for q in range(PK):
                            k = h * PK + q
                            nc.tensor.transpose(
                                xt_ps[:, q * P:(q + 1) * P],
                                x_t[:, k * P:(k + 1) * P],
                                ident[:],
                            )
                        nc.scalar.copy(xT[:, h * PK * P:(h + 1) * PK * P], xt_ps[:])
                    # router logits: [P tokens, E]
                    lg_ps = apsB.tile([P, E], F32, tag="lg")
                    for k in range(KC):
                        nc.tensor.matmul(
                            lg_ps[:],
                            lhsT=xT[:, k * P:(k + 1) * P],
                            rhs=wg_sb[:, k * E:(k + 1) * E],
                            start=(k == 0),
                            stop=False,
                        )
                    nc.tensor.matmul(
                        lg_ps[:], lhsT=ones1[:], rhs=bg_sb[:], start=False, stop=True,
                    )
                    # softmax pieces (no normalization needed for top-k)
                    negm = asb.tile([P, 1], F32, tag="negm")
                    nc.vector.reduce_max(out=negm[:], in_=lg_ps[:], axis=AX.X, negate=True)
                    probs = asb.tile([P, E], F32, tag="probs")
                    nc.scalar.activation(probs[:], lg_ps[:], ACTF.Exp, bias=negm[:])
                    ssum = asb.tile([P, 1], F32, tag="ssum")
                    nc.vector.reduce_sum(out=ssum[:], in_=probs[:], axis=AX.X)
                    rinv = asb.tile([P, 1], F32, tag="rinv")
                    nc.vector.reciprocal(rinv[:], ssum[:])
                    mx8 = asb.tile([P, 8], F32, tag="mx8")
                    nc.vector.max(mx8[:], probs[:])
                    ix8 = asb.tile([P, 8], U32, tag="ix8")
                    nc.vector.max_index(ix8[:], mx8[:], probs[:])
                    # one-hots of the two selected experts, stacked [A | B]
                    ab = asb.tile([P, 16], F32, tag="ab")
                    nc.vector.tensor_scalar(
                        out=ab[:, 0:8], in0=probs[:], scalar1=mx8[:, 0:1],
                        scalar2=None, op0=ALU.is_equal,
                    )
                    nc.vector.tensor_scalar(
                        out=ab[:, 8:16], in0=probs[:], scalar1=mx8[:, 1:2],
                        scalar2=None, op0=ALU.is_equal,
                    )
                    meta = metas[i]
                    nc.vector.tensor_tensor(
                        out=meta[:, 0:1], in0=mx8[:, 0:1], in1=rinv[:], op=ALU.mult)
                    nc.vector.tensor_tensor(
                        out=meta[:, 1:2], in0=mx8[:, 1:2], in1=rinv[:], op=ALU.mult)
                    # transpose A and B -> [8, P] each
                    ab_ps = apsC.tile([8, 2 * P], F32, tag="ab_ps")
                    nc.tensor.transpose(ab_ps[:, 0:P], ab[:, 0:8], ident[:])
                    nc.tensor.transpose(ab_ps[:, P:2 * P], ab[:, 8:16], ident[:])
                    abt = asb.tile([8, 2 * P], F32, tag="abt")
                    nc.scalar.copy(abt[:], ab_ps[:])
                    # chained inclusive cumsum over tokens (per expert)
                    mt = asb.tile([8, P], F32, tag="mt")
                    nc.vector.tensor_tensor(
                        out=mt[:], in0=abt[:, 0:P], in1=abt[:, P:2 * P], op=ALU.add)
                    init = 0.0 if i == 0 else SST[:, i * P - 1:i * P]
                    nc.vector.tensor_tensor_scan(
                        out=SST[:, i * P:(i + 1) * P], data0=mt[:], data1=mt[:],
                        initial=init, op0=ALU.add, op1=ALU.bypass,
                    )
                    # extract this tile's inclusive positions for k=0 / k=1
                    prodt = asb.tile([8, 2 * P], F32, tag="prodt")
                    nc.vector.tensor_tensor(
                        out=prodt[:, 0:P], in0=abt[:, 0:P],
                        in1=SST[:, i * P:(i + 1) * P], op=ALU.mult)
                    nc.vector.tensor_tensor(
                        out=prodt[:, P:2 * P], in0=abt[:, P:2 * P],
                        in1=SST[:, i * P:(i + 1) * P], op=ALU.mult)
                    pos_ps = apsD.tile([1, 2 * P], F32, tag="pos_ps")
                    nc.tensor.matmul(
                        pos_ps[:, 0:P], lhsT=ones8[:], rhs=prodt[:, 0:P],
                        start=True, stop=True,
                    )
                    nc.tensor.matmul(
                        pos_ps[:, P:2 * P], lhsT=ones8[:], rhs=prodt[:, P:2 * P],
                        start=True, stop=True,
                    )
                    posr = asb.tile([1, 2 * P], F32, tag="posr")
                    nc.scalar.copy(posr[:], pos_ps[:])
                    pt_ps = apsD.tile([P, 2], F32, tag="pt_ps")
                    nc.tensor.transpose(pt_ps[:, 0:1], posr[:, 0:P], ident[0:1, 0:1])
                    nc.tensor.transpose(pt_ps[:, 1:2], posr[:, P:2 * P], ident[0:1, 0:1])
                    posT = asb.tile([P, 2], F32, tag="posT")
                    nc.vector.tensor_copy(posT[:], pt_ps[:])
                    keep = asb.tile([P, 2], F32, tag="keep")
                    nc.vector.tensor_scalar(
                        out=keep[:], in0=posT[:], scalar1=float(CAP),
                        scalar2=None, op0=ALU.is_le,
                    )
                    # gates = keep * topk_prob / sum
                    nc.vector.tensor_tensor(
                        out=meta[:, 4:5], in0=meta[:, 0:1], in1=keep[:, 0:1], op=ALU.mult)
                    nc.vector.tensor_tensor(
                        out=meta[:, 5:6], in0=meta[:, 1:2], in1=keep[:, 1:2], op=ALU.mult)
                    # dispatch row in chunk-major layout:
                    #   pos0 = pos_incl - 1, j = pos0 / CH (A2A chunk)
                    #   dst  = j*G + e*CH + pos0 % CH
                    pos_i = asb.tile([P, 2], I32, tag="pos_i")
                    nc.vector.tensor_copy(pos_i[:], posT[:])
                    nc.vector.tensor_scalar(
                        out=pos_i[:], in0=pos_i[:], scalar1=-1,
                        scalar2=None, op0=ALU.add)
                    e_i = asb.tile([P, 2], I32, tag="e_i")
                    nc.vector.tensor_copy(e_i[:, 0:1], ix8[:, 0:1])
                    nc.vector.tensor_copy(e_i[:, 1:2], ix8[:, 1:2])
                    jhi = asb.tile([P, 2], I32, tag="jhi")
                    nc.vector.tensor_scalar(
                        out=jhi[:], in0=pos_i[:], scalar1=SH, scalar2=GSH,
                        op0=ALU.arith_shift_right, op1=ALU.logical_shift_left)
                    dst_i = asb.tile([P, 2], I32, tag="dst_i")
                    nc.vector.tensor_scalar(
                        out=dst_i[:], in0=pos_i[:], scalar1=CH - 1,
                        scalar2=None, op0=ALU.bitwise_and)
                    nc.vector.tensor_tensor(
                        out=dst_i[:], in0=dst_i[:], in1=jhi[:], op=ALU.add)
                    esh = asb.tile([P, 2], I32, tag="esh")
                    nc.vector.tensor_scalar(
                        out=esh[:], in0=e_i[:], scalar1=SH,
                        scalar2=None, op0=ALU.logical_shift_left)
                    nc.vector.tensor_tensor(
                        out=dst_i[:], in0=dst_i[:], in1=esh[:], op=ALU.add)
                    keep_i = asb.tile([P, 2], I32, tag="keep_i")
                    nc.vector.tensor_copy(keep_i[:], keep[:])
                    idx = idxs[i]
                    nc.vector.memset(idx[:, 0:2], ECAP)       # dropped -> OOB, skipped
                    nc.vector.copy_predicated(idx[:, 0:2], keep_i[:], dst_i[:])
                    nc.vector.memset(idx[:, 2:4], 0)          # dropped -> row 0, gate 0
                    nc.vector.copy_predicated(idx[:, 2:4], keep_i[:], dst_i[:])
                    # dispatch scatter for this tile (both k-slots)
                    for k in range(2):
                        nc.gpsimd.indirect_dma_start(
                            out=disp[:, :],
                            out_offset=IndirectOffsetOnAxis(ap=idx[:, k:k + 1], axis=0),
                            in_=x_t[:, :],
                            in_offset=None,
                            bounds_check=ECAP - 1,
                            oob_is_err=False,
                        )
            # ================= Phase C: AllToAll (dispatch), chunked ======
            for j in range(NG):
                nc.gpsimd.collective_compute(
                    "AllToAll", ALU.bypass, replica_groups=[cores],
                    ins=[disp[j * G:(j + 1) * G, :].opt()],
                    outs=[recv[j * G:(j + 1) * G, :].opt()],
                )

            # ================= Phase D: expert FFN ========================
            half1 = dt_mm1 in (mybir.dt.float16, mybir.dt.bfloat16)
            half2 = dt_mm2 in (mybir.dt.float16, mybir.dt.bfloat16)
            with (
                tc.tile_pool(name="frecv", bufs=NS + 2 if half1 else NS + 1) as frecv,
                tc.tile_pool(name="fw1", bufs=8 if half1 else 4) as fw1,
                tc.tile_pool(name="fw2", bufs=3 if half2 else 2) as fw2,
                tc.tile_pool(name="ftokT", bufs=2) as ftokT,
                tc.tile_pool(name="fhT", bufs=2 if half2 else 1) as fhT,
                tc.tile_pool(name="fyT", bufs=2 if half2 else 1) as fyT,
                tc.tile_pool(name="fy", bufs=4) as fy,
                tc.tile_pool(name="fps_t", bufs=2, space="PSUM") as fps_t,
                tc.tile_pool(name="fps_h", bufs=2, space="PSUM") as fps_h,
                tc.tile_pool(name="fps_y", bufs=2, space="PSUM") as fps_y,
                tc.tile_pool(name="fps_o", bufs=2, space="PSUM") as fps_o,
            ):
                cast_tok = dt_mm1 in (mybir.dt.float16, mybir.dt.bfloat16)
                identh = None
                if cast_tok:
                    identh = constp.tile([P, P], dt_mm1, name="identh")
                    nc.vector.tensor_copy(identh[:], ident[:])
                for g in range(NG):
                    rts = []
                    for s in range(NS):
                        rt = frecv.tile([P, C], F32, tag="rt")
                        nc.scalar.dma_start(
                            rt[:], recv[(g * NS + s) * P:(g * NS + s + 1) * P, :])
                        if cast_tok:
                            # pre-cast on the idle DVE: the matmul would round
                            # to dt_mm1 anyway, and 16-bit PE transposes run 2x
                            rth = frecv.tile([P, C], dt_mm1, tag="rth")
                            nc.vector.tensor_copy(rth[:], rt[:])
                            rts.append(rth)
                        else:
                            rts.append(rt)
                    tokT = ftokT.tile([P, KC * G], dt_mm1, tag="tokT")
                    for k in range(KC):
                        tp = fps_t.tile([P, G], dt_mm1 if cast_tok else F32, tag="tp")
                        for s in range(NS):
                            nc.tensor.transpose(
                                tp[:, s * P:(s + 1) * P],
                                rts[s][:, k * P:(k + 1) * P],
                                identh[:] if cast_tok else ident[:],
                            )
                        nc.scalar.copy(tokT[:, k * G:(k + 1) * G], tp[:])
                    hT = fhT.tile([P, KD * G], dt_mm2, tag="hT")
                    for m in range(KD):
                        w1g = fw1.tile([P, KC * P], dt_mm1, tag="w1g")
                        nc.sync.dma_start(w1g[:], w1_ext[m])
                        hp = fps_h.tile([P, G], F32, tag="hp")
                        for k in range(KC):
                            nc.tensor.matmul(
                                hp[:], lhsT=w1g[:, k * P:(k + 1) * P],
                                rhs=tokT[:, k * G:(k + 1) * G],
                                start=(k == 0), stop=(k == KC - 1),
                            )
                        nc.scalar.activation(
                            hT[:, m * G:(m + 1) * G], hp[:], ACTF.Relu,
                            bias=b1_sb[:, m:m + 1],
                        )
                    yT = fyT.tile([P, KC * G], F32, tag="yT")
                    for mc in range(KC):
                        w2g = fw2.tile([P, KD * P], dt_mm2, tag="w2g")
                        nc.sync.dma_start(w2g[:], w2_ext[mc])
                        yp = fps_y.tile([P, G], F32, tag="yp")
                        for k in range(KD):
                            nc.tensor.matmul(
                                yp[:], lhsT=w2g[:, k * P:(k + 1) * P],
                                rhs=hT[:, k * G:(k + 1) * G],
                                start=(k == 0), stop=(k == KD - 1),
                            )
                        nc.scalar.activation(
                            yT[:, mc * G:(mc + 1) * G], yp[:], ACTF.Identity,
                            bias=b2_sb[:, mc:mc + 1],
                        )
                    # transpose back to [tokens, C] and store
                    PK = min(4, KC)
                    for s in range(NS):
                        y_t = fy.tile([P, C], F32, tag="y_t")
                        for h in range(KC // PK):
                            op_ps = fps_o.tile([P, PK * P], F32, tag="op_ps")
                            for q in range(PK):
                                mc = h * PK + q
                                nc.tensor.transpose(
                                    op_ps[:, q * P:(q + 1) * P],
                                    yT[:, mc * G + s * P: mc * G + (s + 1) * P],
                                    ident[:],
                                )
                            nc.scalar.copy(y_t[:, h * PK * P:(h + 1) * PK * P], op_ps[:])
                        nc.scalar.dma_start(
                            ysend[(g * NS + s) * P:(g * NS + s + 1) * P, :], y_t[:])

            # ================= Phase E: AllToAll (combine), chunked =======
            for j in range(NG):
                nc.gpsimd.collective_compute(
                    "AllToAll", ALU.bypass, replica_groups=[cores],
                    ins=[ysend[j * G:(j + 1) * G, :].opt()],
                    outs=[recv2[j * G:(j + 1) * G, :].opt()],
                )

            with (
                tc.tile_pool(name="cg", bufs=12) as cgp,
            ):
                for i in range(NT):
                    g0 = cgp.tile([P, C], F32, tag="g0")
                    nc.gpsimd.indirect_dma_start(
                        out=g0[:, :], out_offset=None,
                        in_=recv2[:, :],
                        in_offset=IndirectOffsetOnAxis(ap=idxs[i][:, 2:3], axis=0),
                    )
                    g1 = cgp.tile([P, C], F32, tag="g1")
                    nc.gpsimd.indirect_dma_start(
                        out=g1[:, :], out_offset=None,
                        in_=recv2[:, :],
                        in_offset=IndirectOffsetOnAxis(ap=idxs[i][:, 3:4], axis=0),
                    )
                    o_t = cgp.tile([P, C], F32, tag="o_t")
                    nc.scalar.activation(
                        o_t[:], g0[:], ACTF.Copy, scale=metas[i][:, 4:5])
                    g1s = cgp.tile([P, C], F32, tag="g1s")
                    nc.vector.tensor_scalar(
                        out=g1s[:], in0=g1[:], scalar1=metas[i][:, 5:6],
                        scalar2=None, op0=ALU.mult,
                    )
                    nc.vector.tensor_tensor(out=o_t[:], in0=o_t[:], in1=g1s[:], op=ALU.add)
                    nc.scalar.dma_start(out_ext[i * P:(i + 1) * P, :], o_t[:])

    nc.compile()
    return nc


# ---------------------------------------------------------------------------
# Host-side entry point
# ---------------------------------------------------------------------------

_NC_CACHE = {}


def _get_nc(key, **kw):
    if key not in _NC_CACHE:
        _NC_CACHE[key] = build_moe_nc(**kw)
    return _NC_CACHE[key]


def prep_inputs(x, Wg, bg, W1, b1, W2, b2, dt_np1=np.float32, dt_np2=np.float32):
    """Build the per-core input maps (host-side sharding / weight tiling)."""
    B, T, C = x.shape
    E, _, DFF = W1.shape
    KC, KD = C // P, DFF // P
    wgt = np.ascontiguousarray(
        np.asarray(Wg, np.float32).reshape(KC, P, E).transpose(1, 0, 2))
    bgr = np.asarray(bg, np.float32).reshape(1, E)
    in_maps = []
    for b in range(B):
        w1t = np.ascontiguousarray(
            np.asarray(W1[b], dt_np1).reshape(KC, P, KD, P).transpose(2, 1, 0, 3)
        ).reshape(KD, P, KC * P)
        w2t = np.ascontiguousarray(
            np.asarray(W2[b], dt_np2).reshape(KD, P, KC, P).transpose(2, 1, 0, 3)
        ).reshape(KC, P, KD * P)
        b1t = np.ascontiguousarray(np.asarray(b1[b], np.float32).reshape(KD, P).T)
        b2t = np.ascontiguousarray(np.asarray(b2[b], np.float32).reshape(KC, P).T)
        in_maps.append({
            "x": np.ascontiguousarray(np.asarray(x[b], np.float32)),
            "wgt": wgt, "bg": bgr,
            "w1t": w1t, "b1t": b1t, "w2t": w2t, "b2t": b2t,
        })
    return in_maps


def run_moe(x, Wg, bg, W1, b1, W2, b2, dt_mm1=F32, dt_mm2=F32, trace=False):
    B, T, C = x.shape
    E, _, DFF = W1.shape
    CAP = int(T / E * 1.25)
    nc = _get_nc((T, C, E, CAP, DFF, dt_mm1, dt_mm2),
                 T=T, C=C, E=E, CAP=CAP, DFF=DFF, dt_mm1=dt_mm1, dt_mm2=dt_mm2)

    def np_of(d):
        return np.float32 if d in (F32, mybir.dt.float32r) else mybir.dt.np(d)

    in_maps = prep_inputs(x, Wg, bg, W1, b1, W2, b2,
                          dt_np1=np_of(dt_mm1), dt_np2=np_of(dt_mm2))
    res = run_bass_kernel_spmd(nc, in_maps, list(range(E)), trace=trace)
    out = np.stack([res.results[b]["out"] for b in range(B)], axis=0)
    return out, res


DEFAULT_DT1 = mybir.dt.float16
DEFAULT_DT2 = mybir.dt.float16


def kernel(x, Wg, bg, W1, b1, W2, b2):
    out, _ = run_moe(
        np.asarray(x), np.asarray(Wg), np.asarray(bg), np.asarray(W1),
        np.asarray(b1), np.asarray(W2), np.asarray(b2),
        dt_mm1=DEFAULT_DT1, dt_mm2=DEFAULT_DT2,
    )
    return out


# revision 35
# speedup vs baseline: 1.0195x; 1.0195x over previous
"""Distributed MoE (top-2 routing, capacity 320) on 8 Trainium2 NeuronCores.

Sharding (matches the expert-parallel hint):
  - x is data-parallel sharded along B: core b owns batch row b (2048 tokens).
  - W1/b1/W2/b2 are sharded along the expert dim: core e owns expert e.
  - The router (Wg, bg) is replicated; each core routes its own tokens.
  - Dispatch: each core scatters its tokens into a [E, CAP, C] buffer and an
    AllToAll moves expert-e slabs to core e, which then holds [B, CAP, C]
    tokens for its expert. After the expert FFN a second AllToAll returns
    [E, CAP, C] outputs to each data-parallel core, which combines them with
    the gate probabilities.

Everything (router matmul, softmax, top-2, capacity positions via a prefix
scan, scatter/gather via indirect DMA, the two AllToAlls, and the expert FFN)
runs on-device; the host only slices/reassembles numpy arrays.

Key implementation points:
  - The dispatch buffer uses a chunk-major layout (row = j*G + e*CH + pos%CH,
    j = pos//CH) so each AllToAll is split into NG=5 chunked collectives whose
    transfers overlap the expert FFN groups (group g consumes chunk g).
  - Expert weights are passed host-pre-tiled as [out_chunk, 128, K*128] so one
    DMA per 128-wide output chunk loads all contraction tiles with contiguous
    16KB partition lines (the naive per-tile layout saturated the in-order
    sync sequencer and starved the PE).
  - FFN output DMAs ride the ACT HWDGE ring so they never block the sync ring
    that streams weights.
  - The token-position cumsum is a chained `tensor_tensor_scan`, fused per
    token tile so routing, index build, and dispatch scatter pipeline.
  - Matmuls default to fp16 (1 cycle/row vs 4 for fp32's two half-speed
    passes; measured rel. error 4.2e-4 vs the f32 reference, routing/top-k
    decisions are computed in exact f32 and match the reference bit-for-bit).
"""

import numpy as np

import concourse.mybir as mybir
import concourse.tile as tile
from concourse import bacc
from concourse.bass import IndirectOffsetOnAxis
from concourse.bass_utils import run_bass_kernel_spmd
from concourse.masks import make_identity

F32 = mybir.dt.float32
I32 = mybir.dt.int32
U32 = mybir.dt.uint32
AX = mybir.AxisListType
ALU = mybir.AluOpType
ACTF = mybir.ActivationFunctionType

P = 128


def build_moe_nc(T=2048, C=1024, E=8, CAP=320, DFF=4096, dt_mm1=F32, dt_mm2=F32, zero_disp=False):
    """Build the per-core (SPMD) Bass program. All 8 cores run this module."""
    assert T % P == 0 and C % P == 0 and DFF % P == 0
    NT = T // P         # token tiles per core
    KC = C // P         # C chunks (contraction for matmul1)
    KD = DFF // P       # DFF chunks (contraction for matmul2)
    ECAP = E * CAP      # rows in the dispatch buffer
    G = 512 if ECAP % 512 == 0 else ECAP   # FFN token-group size / A2A chunk rows
    assert ECAP % G == 0 and G % P == 0
    NG = ECAP // G      # FFN groups == A2A chunks
    NS = G // P         # 128-token subtiles per group
    CH = G // E         # capacity rows per (expert, chunk)
    SH = CH.bit_length() - 1
    assert (1 << SH) == CH, "chunk size must be a power of two"
    GSH = G.bit_length() - 1
    assert (1 << GSH) == G, "group size must be a power of two"
    assert CAP == NG * CH
    cores = list(range(E))

    nc = bacc.Bacc(None, target_bir_lowering=False, debug=False)

    # ---- I/O (per core) --------------------------------------------------
    x_ext = nc.dram_tensor("x", [T, C], F32, kind="ExternalInput")
    wg_ext = nc.dram_tensor("wgt", [P, KC, E], F32, kind="ExternalInput")   # Wg[C,E] -> [P, KC, E]
    bg_ext = nc.dram_tensor("bg", [1, E], F32, kind="ExternalInput")
    w1_ext = nc.dram_tensor("w1t", [KD, P, KC * P], dt_mm1, kind="ExternalInput")
    b1_ext = nc.dram_tensor("b1t", [P, KD], F32, kind="ExternalInput")
    w2_ext = nc.dram_tensor("w2t", [KC, P, KD * P], dt_mm2, kind="ExternalInput")
    b2_ext = nc.dram_tensor("b2t", [P, KC], F32, kind="ExternalInput")
    out_ext = nc.dram_tensor("out", [T, C], F32, kind="ExternalOutput")

    with tile.TileContext(nc) as tc:
        with (
            tc.tile_pool(name="const", bufs=1) as constp,
            tc.tile_pool(name="dram", bufs=1, space="DRAM") as dramp,
            tc.tile_pool(name="route", bufs=1) as routep,
        ):
            # ---- internal DRAM (collective + staging buffers) ----
            disp = dramp.tile([ECAP, C], F32)    # my tokens, per-expert slabs
            recv = dramp.tile([ECAP, C], F32)    # post-A2A: my expert, per-src slabs
            ysend = dramp.tile([ECAP, C], F32)   # expert outputs, per-src slabs
            recv2 = dramp.tile([ECAP, C], F32)   # post-A2A: my tokens' expert outputs

            # ---- constants ----
            ident = constp.tile([P, P], F32)
            make_identity(nc, ident)
            wg_sb = constp.tile([P, KC * E], F32)
            nc.sync.dma_start(wg_sb[:], wg_ext[:])
            bg_sb = constp.tile([1, E], F32)
            nc.sync.dma_start(bg_sb[:], bg_ext[:])
            ones1 = constp.tile([1, P], F32)
            nc.vector.memset(ones1[:], 1.0)
            ones8 = constp.tile([8, 1], F32)
            nc.vector.memset(ones8[:], 1.0)
            b1_sb = constp.tile([P, KD], F32)
            nc.sync.dma_start(b1_sb[:], b1_ext[:])
            b2_sb = constp.tile([P, KC], F32)
            nc.sync.dma_start(b2_sb[:], b2_ext[:])

            # ---- persistent routing tables (small; survive into combine) ----
            metas = [routep.tile([P, 8], F32, tag=f"meta{i}", name=f"meta{i}") for i in range(NT)]
            idxs = [routep.tile([P, 4], I32, tag=f"idx{i}", name=f"idx{i}") for i in range(NT)]

            # ================= Phase A: router + top-2 ====================
            with (
                tc.tile_pool(name="xa", bufs=1) as xap,
                tc.tile_pool(name="xtp", bufs=4) as xtp,
                tc.tile_pool(name="apsA", bufs=2, space="PSUM") as apsA,
                tc.tile_pool(name="apsB", bufs=2, space="PSUM") as apsB,
                tc.tile_pool(name="apsC", bufs=2, space="PSUM") as apsC,
                tc.tile_pool(name="apsD", bufs=1, space="PSUM") as apsD,
                tc.tile_pool(name="asb", bufs=4) as asb,
                tc.tile_pool(name="ascr", bufs=1) as ascr,
            ):
                # phase-A scratch (freed before the FFN needs the SBUF)
                SST = ascr.tile([8, T], F32)          # chained cumsum of expert one-hots
                if zero_disp:
                    # unfilled capacity slots never reach the output; zeroing
                    # only satisfies the simulator's NaN checker (emitted
                    # before the scatters, ordered via Tile WAW deps)
                    zt = asb.tile([P, C], F32, tag="zt", bufs=1)
                    nc.vector.memset(zt[:], 0.0)
                    for j in range(ECAP // P):
                        nc.gpsimd.dma_start(disp[j * P:(j + 1) * P, :], zt[:])
                x_tiles = []
                for i in range(NT):
                    x_t = xap.tile([P, C], F32, tag=f"x{i}", name=f"x{i}")
                    x_tiles.append(x_t)
                    nc.sync.dma_start(x_t[:], x_ext[i * P:(i + 1) * P, :])
                    # transpose x tile -> xT (C on partitions)
                    xT = xtp.tile([P, C], F32, tag="xT")
                    PK = min(4, KC)
                    for h in range(KC // PK):
                        xt_ps = apsA.tile([P, PK * P], F32, tag="xt_ps")
                        for q in range(PK):
                            k = h * PK + q
                            nc.tensor.transpose(
                                xt_ps[:, q * P:(q + 1) * P],
                                x_t[:, k * P:(k + 1) * P],
                                ident[:],
                            )
                        nc.scalar.copy(xT[:, h * PK * P:(h + 1) * PK * P], xt_ps[:])
                    # router logits: [P tokens, E]
                    lg_ps = apsB.tile([P, E], F32, tag="lg")
                    for k in range(KC):
                        nc.tensor.matmul(
                            lg_ps[:],
                            lhsT=xT[:, k * P:(k + 1) * P],
                            rhs=wg_sb[:, k * E:(k + 1) * E],
                            start=(k == 0),
                            stop=False,
                        )
                    nc.tensor.matmul(
                        lg_ps[:], lhsT=ones1[:], rhs=bg_sb[:], start=False, stop=True,
                    )
                    # softmax pieces (no normalization needed for top-k)
                    negm = asb.tile([P, 1], F32, tag="negm")
                    nc.vector.reduce_max(out=negm[:], in_=lg_ps[:], axis=AX.X, negate=True)
                    probs = asb.tile([P, E], F32, tag="probs")
                    nc.scalar.activation(probs[:], lg_ps[:], ACTF.Exp, bias=negm[:])
                    ssum = asb.tile([P, 1], F32, tag="ssum")
                    nc.vector.reduce_sum(out=ssum[:], in_=probs[:], axis=AX.X)
                    rinv = asb.tile([P, 1], F32, tag="rinv")
                    nc.vector.reciprocal(rinv[:], ssum[:])
                    mx8 = asb.tile([P, 8], F32, tag="mx8")
                    nc.vector.max(mx8[:], probs[:])
                    ix8 = asb.tile([P, 8], U32, tag="ix8")
                    nc.vector.max_index(ix8[:], mx8[:], probs[:])
                    # one-hots of the two selected experts, stacked [A | B]
                    ab = asb.tile([P, 16], F32, tag="ab")
                    nc.vector.tensor_scalar(
                        out=ab[:, 0:8], in0=probs[:], scalar1=mx8[:, 0:1],
                        scalar2=None, op0=ALU.is_equal,
                    )
                    nc.vector.tensor_scalar(
                        out=ab[:, 8:16], in0=probs[:], scalar1=mx8[:, 1:2],
                        scalar2=None, op0=ALU.is_equal,
                    )
                    meta = metas[i]
                    nc.vector.tensor_tensor(
                        out=meta[:, 0:1], in0=mx8[:, 0:1], in1=rinv[:], op=ALU.mult)
                    nc.vector.tensor_tensor(
                        out=meta[:, 1:2], in0=mx8[:, 1:2], in1=rinv[:], op=ALU.mult)
                    # transpose A and B -> [8, P] each
                    ab_ps = apsC.tile([8, 2 * P], F32, tag="ab_ps")
                    nc.tensor.transpose(ab_ps[:, 0:P], ab[:, 0:8], ident[:])
                    nc.tensor.transpose(ab_ps[:, P:2 * P], ab[:, 8:16], ident[:])
                    abt = asb.tile([8, 2 * P], F32, tag="abt")
                    nc.scalar.copy(abt[:], ab_ps[:])
                    # chained inclusive cumsum over tokens (per expert)
                    mt = asb.tile([8, P], F32, tag="mt")
                    nc.vector.tensor_tensor(
                        out=mt[:], in0=abt[:, 0:P], in1=abt[:, P:2 * P], op=ALU.add)
                    init = 0.0 if i == 0 else SST[:, i * P - 1:i * P]
                    nc.vector.tensor_tensor_scan(
                        out=SST[:, i * P:(i + 1) * P], data0=mt[:], data1=mt[:],
                        initial=init, op0=ALU.add, op1=ALU.bypass,
                    )
                    # extract this tile's inclusive positions for k=0 / k=1
                    prodt = asb.tile([8, 2 * P], F32, tag="prodt")
                    nc.vector.tensor_tensor(
                        out=prodt[:, 0:P], in0=abt[:, 0:P],
                        in1=SST[:, i * P:(i + 1) * P], op=ALU.mult)
                    nc.vector.tensor_tensor(
                        out=prodt[:, P:2 * P], in0=abt[:, P:2 * P],
                        in1=SST[:, i * P:(i + 1) * P], op=ALU.mult)
                    pos_ps = apsD.tile([1, 2 * P], F32, tag="pos_ps")
                    nc.tensor.matmul(
                        pos_ps[:, 0:P], lhsT=ones8[:], rhs=prodt[:, 0:P],
                        start=True, stop=True,
                    )
                    nc.tensor.matmul(
                        pos_ps[:, P:2 * P], lhsT=ones8[:], rhs=prodt[:, P:2 * P],
                        start=True, stop=True,
                    )
                    posr = asb.tile([1, 2 * P], F32, tag="posr")
                    nc.scalar.copy(posr[:], pos_ps[:])
                    pt_ps = apsD.tile([P, 2], F32, tag="pt_ps")
                    nc.tensor.transpose(pt_ps[:, 0:1], posr[:, 0:P], ident[0:1, 0:1])
                    nc.tensor.transpose(pt_ps[:, 1:2], posr[:, P:2 * P], ident[0:1, 0:1])
                    posT = asb.tile([P, 2], F32, tag="posT")
                    nc.vector.tensor_copy(posT[:], pt_ps[:])
                    keep = asb.tile([P, 2], F32, tag="keep")
                    nc.vector.tensor_scalar(
                        out=keep[:], in0=posT[:], scalar1=float(CAP),
                        scalar2=None, op0=ALU.is_le,
                    )
                    # gates = keep * topk_prob / sum
                    nc.vector.tensor_tensor(
                        out=meta[:, 4:5], in0=meta[:, 0:1], in1=keep[:, 0:1], op=ALU.mult)
                    nc.vector.tensor_tensor(
                        out=meta[:, 5:6], in0=meta[:, 1:2], in1=keep[:, 1:2], op=ALU.mult)
                    # dispatch row in chunk-major layout:
                    #   pos0 = pos_incl - 1, j = pos0 / CH (A2A chunk)
                    #   dst  = j*G + e*CH + pos0 % CH
                    pos_i = asb.tile([P, 2], I32, tag="pos_i")
                    nc.vector.tensor_copy(pos_i[:], posT[:])
                    nc.vector.tensor_scalar(
                        out=pos_i[:], in0=pos_i[:], scalar1=-1,
                        scalar2=None, op0=ALU.add)
                    e_i = asb.tile([P, 2], I32, tag="e_i")
                    nc.vector.tensor_copy(e_i[:, 0:1], ix8[:, 0:1])
                    nc.vector.tensor_copy(e_i[:, 1:2], ix8[:, 1:2])
                    jhi = asb.tile([P, 2], I32, tag="jhi")
                    nc.vector.tensor_scalar(
                        out=jhi[:], in0=pos_i[:], scalar1=SH, scalar2=GSH,
                        op0=ALU.arith_shift_right, op1=ALU.logical_shift_left)
                    dst_i = asb.tile([P, 2], I32, tag="dst_i")
                    nc.vector.tensor_scalar(
                        out=dst_i[:], in0=pos_i[:], scalar1=CH - 1,
                        scalar2=None, op0=ALU.bitwise_and)
                    nc.vector.tensor_tensor(
                        out=dst_i[:], in0=dst_i[:], in1=jhi[:], op=ALU.add)
                    esh = asb.tile([P, 2], I32, tag="esh")
                    nc.vector.tensor_scalar(
                        out=esh[:], in0=e_i[:], scalar1=SH,
                        scalar2=None, op0=ALU.logical_shift_left)
                    nc.vector.tensor_tensor(
                        out=dst_i[:], in0=dst_i[:], in1=esh[:], op=ALU.add)
                    keep_i = asb.tile([P, 2], I32, tag="keep_i")
                    nc.vector.tensor_copy(keep_i[:], keep[:])
                    idx = idxs[i]
                    nc.vector.memset(idx[:, 0:2], ECAP)       # dropped -> OOB, skipped
                    nc.vector.copy_predicated(idx[:, 0:2], keep_i[:], dst_i[:])
                    nc.vector.memset(idx[:, 2:4], 0)          # dropped -> row 0, gate 0
                    nc.vector.copy_predicated(idx[:, 2:4], keep_i[:], dst_i[:])
                    # dispatch scatter for this tile (both k-slots)
                    for k in range(2):
                        nc.gpsimd.indirect_dma_start(
                            out=disp[:, :],
                            out_offset=IndirectOffsetOnAxis(ap=idx[:, k:k + 1], axis=0),
                            in_=x_t[:, :],
                            in_offset=None,
                            bounds_check=ECAP - 1,
                            oob_is_err=False,
                        )
            # ================= Phase C: AllToAll (dispatch), chunked ======
            for j in range(NG):
                nc.gpsimd.collective_compute(
                    "AllToAll", ALU.bypass, replica_groups=[cores],
                    ins=[disp[j * G:(j + 1) * G, :].opt()],
                    outs=[recv[j * G:(j + 1) * G, :].opt()],
                )

            # ================= Phase D: expert FFN ========================
            half1 = dt_mm1 in (mybir.dt.float16, mybir.dt.bfloat16)
            half2 = dt_mm2 in (mybir.dt.float16, mybir.dt.bfloat16)
            with (
                tc.tile_pool(name="frecv", bufs=NS + 2 if half1 else NS + 1) as frecv,
                tc.tile_pool(name="fw1", bufs=8 if half1 else 4) as fw1,
                tc.tile_pool(name="fw2", bufs=3 if half2 else 2) as fw2,
                tc.tile_pool(name="ftokT", bufs=2) as ftokT,
                tc.tile_pool(name="fhT", bufs=2 if half2 else 1) as fhT,
                tc.tile_pool(name="fyT", bufs=2 if half2 else 1) as fyT,
                tc.tile_pool(name="fy", bufs=4) as fy,
                tc.tile_pool(name="fps_t", bufs=2, space="PSUM") as fps_t,
                tc.tile_pool(name="fps_h", bufs=2, space="PSUM") as fps_h,
                tc.tile_pool(name="fps_y", bufs=2, space="PSUM") as fps_y,
                tc.tile_pool(name="fps_o", bufs=2, space="PSUM") as fps_o,
            ):
                cast_tok = dt_mm1 in (mybir.dt.float16, mybir.dt.bfloat16)
                identh = None
                if cast_tok:
                    identh = constp.tile([P, P], dt_mm1, name="identh")
                    nc.vector.tensor_copy(identh[:], ident[:])
                for g in range(NG):
                    rts = []
                    for s in range(NS):
                        rt = frecv.tile([P, C], F32, tag="rt")
                        nc.scalar.dma_start(
                            rt[:], recv[(g * NS + s) * P:(g * NS + s + 1) * P, :])
                        if cast_tok:
                            # pre-cast on the idle DVE: the matmul would round
                            # to dt_mm1 anyway, and 16-bit PE transposes run 2x
                            rth = frecv.tile([P, C], dt_mm1, tag="rth")
                            nc.vector.tensor_copy(rth[:], rt[:])
                            rts.append(rth)
                        else:
                            rts.append(rt)
                    tokT = ftokT.tile([P, KC * G], dt_mm1, tag="tokT")
                    for k in range(KC):
                        tp = fps_t.tile([P, G], dt_mm1 if cast_tok else F32, tag="tp")
                        for s in range(NS):
                            nc.tensor.transpose(
                                tp[:, s * P:(s + 1) * P],
                                rts[s][:, k * P:(k + 1) * P],
                                identh[:] if cast_tok else ident[:],
                            )
                        nc.scalar.copy(tokT[:, k * G:(k + 1) * G], tp[:])
                    hT = fhT.tile([P, KD * G], dt_mm2, tag="hT")
                    for m in range(KD):
                        w1g = fw1.tile([P, KC * P], dt_mm1, tag="w1g")
                        nc.sync.dma_start(w1g[:], w1_ext[m])
                        hp = fps_h.tile([P, G], F32, tag="hp")
                        for k in range(KC):
                            nc.tensor.matmul(
                                hp[:], lhsT=w1g[:, k * P:(k + 1) * P],
                                rhs=tokT[:, k * G:(k + 1) * G],
                                start=(k == 0), stop=(k == KC - 1),
                            )
                        nc.scalar.activation(
                            hT[:, m * G:(m + 1) * G], hp[:], ACTF.Relu,
                            bias=b1_sb[:, m:m + 1],
                        )
                    yT = fyT.tile([P, KC * G], F32, tag="yT")
                    for mc in range(KC):
                        w2g = fw2.tile([P, KD * P], dt_mm2, tag="w2g")
                        nc.sync.dma_start(w2g[:], w2_ext[mc])
                        yp = fps_y.tile([P, G], F32, tag="yp")
                        for k in range(KD):
                            nc.tensor.matmul(
                                yp[:], lhsT=w2g[:, k * P:(k + 1) * P],
                                rhs=hT[:, k * G:(k + 1) * G],
                                start=(k == 0), stop=(k == KD - 1),
                            )
                        nc.scalar.activation(
                            yT[:, mc * G:(mc + 1) * G], yp[:], ACTF.Identity,
                            bias=b2_sb[:, mc:mc + 1],
                        )
                    # transpose back to [tokens, C] and store
                    PK = min(4, KC)
                    for s in range(NS):
                        y_t = fy.tile([P, C], F32, tag="y_t")
                        for h in range(KC // PK):
                            op_ps = fps_o.tile([P, PK * P], F32, tag="op_ps")
                            for q in range(PK):
                                mc = h * PK + q
                                nc.tensor.transpose(
                                    op_ps[:, q * P:(q + 1) * P],
                                    yT[:, mc * G + s * P: mc * G + (s + 1) * P],
                                    ident[:],
                                )
                            nc.scalar.copy(y_t[:, h * PK * P:(h + 1) * PK * P], op_ps[:])
                        nc.scalar.dma_start(
                            ysend[(g * NS + s) * P:(g * NS + s + 1) * P, :], y_t[:])

            # ================= Phase E: AllToAll (combine), chunked =======
            for j in range(NG):
                nc.gpsimd.collective_compute(
                    "AllToAll", ALU.bypass, replica_groups=[cores],
                    ins=[ysend[j * G:(j + 1) * G, :].opt()],
                    outs=[recv2[j * G:(j + 1) * G, :].opt()],
                )

            with (
                tc.tile_pool(name="cg", bufs=12) as cgp,
            ):
                for i in range(NT):
                    g0 = cgp.tile([P, C], F32, tag="g0")
                    nc.gpsimd.indirect_dma_start(
                        out=g0[:, :], out_offset=None,
                        in_=recv2[:, :],
                        in_offset=IndirectOffsetOnAxis(ap=idxs[i][:, 2:3], axis=0),
                    )
                    g1 = cgp.tile([P, C], F32, tag="g1")
                    nc.gpsimd.indirect_dma_start(
                        out=g1[:, :], out_offset=None,
                        in_=recv2[:, :],
                        in_offset=IndirectOffsetOnAxis(ap=idxs[i][:, 3:4], axis=0),
                    )
                    o_t = cgp.tile([P, C], F32, tag="o_t")
                    nc.scalar.activation(
                        o_t[:], g0[:], ACTF.Copy, scale=metas[i][:, 4:5])
                    g1s = cgp.tile([P, C], F32, tag="g1s")
                    nc.vector.tensor_scalar(
                        out=g1s[:], in0=g1[:], scalar1=metas[i][:, 5:6],
                        scalar2=None, op0=ALU.mult,
                    )
                    nc.vector.tensor_tensor(out=o_t[:], in0=o_t[:], in1=g1s[:], op=ALU.add)
                    nc.scalar.dma_start(out_ext[i * P:(i + 1) * P, :], o_t[:])

    nc.compile()
    return nc


# ---------------------------------------------------------------------------
# Host-side entry point
# ---------------------------------------------------------------------------

_NC_CACHE = {}


def _get_nc(key, **kw):
    if key not in _NC_CACHE:
        _NC_CACHE[key] = build_moe_nc(**kw)
    return _NC_CACHE[key]


def prep_inputs(x, Wg, bg, W1, b1, W2, b2, dt_np1=np.float32, dt_np2=np.float32):
    """Build the per-core input maps (host-side sharding / weight tiling)."""
    B, T, C = x.shape
    E, _, DFF = W1.shape
    KC, KD = C // P, DFF // P
    wgt = np.ascontiguousarray(
        np.asarray(Wg, np.float32).reshape(KC, P, E).transpose(1, 0, 2))
    bgr = np.asarray(bg, np.float32).reshape(1, E)
    in_maps = []
    for b in range(B):
        w1t = np.ascontiguousarray(
            np.asarray(W1[b], dt_np1).reshape(KC, P, KD, P).transpose(2, 1, 0, 3)
        ).reshape(KD, P, KC * P)
        w2t = np.ascontiguousarray(
            np.asarray(W2[b], dt_np2).reshape(KD, P, KC, P).transpose(2, 1, 0, 3)
        ).reshape(KC, P, KD * P)
        b1t = np.ascontiguousarray(np.asarray(b1[b], np.float32).reshape(KD, P).T)
        b2t = np.ascontiguousarray(np.asarray(b2[b], np.float32).reshape(KC, P).T)
        in_maps.append({
            "x": np.ascontiguousarray(np.asarray(x[b], np.float32)),
            "wgt": wgt, "bg": bgr,
            "w1t": w1t, "b1t": b1t, "w2t": w2t, "b2t": b2t,
        })
    return in_maps


def run_moe(x, Wg, bg, W1, b1, W2, b2, dt_mm1=F32, dt_mm2=F32, trace=False):
    B, T, C = x.shape
    E, _, DFF = W1.shape
    CAP = int(T / E * 1.25)
    nc = _get_nc((T, C, E, CAP, DFF, dt_mm1, dt_mm2),
                 T=T, C=C, E=E, CAP=CAP, DFF=DFF, dt_mm1=dt_mm1, dt_mm2=dt_mm2)

    def np_of(d):
        return np.float32 if d in (F32, mybir.dt.float32r) else mybir.dt.np(d)

    in_maps = prep_inputs(x, Wg, bg, W1, b1, W2, b2,
                          dt_np1=np_of(dt_mm1), dt_np2=np_of(dt_mm2))
    res = run_bass_kernel_spmd(nc, in_maps, list(range(E)), trace=trace)
    out = np.stack([res.results[b]["out"] for b in range(B)], axis=0)
    return out, res


DEFAULT_DT1 = mybir.dt.float16
DEFAULT_DT2 = mybir.dt.float16


def kernel(x, Wg, bg, W1, b1, W2, b2):
    out, _ = run_moe(
        np.asarray(x), np.asarray(Wg), np.asarray(bg), np.asarray(W1),
        np.asarray(b1), np.asarray(W2), np.asarray(b2),
        dt_mm1=DEFAULT_DT1, dt_mm2=DEFAULT_DT2,
    )
    return out


# revision 37
# speedup vs baseline: 1.0195x; 1.0000x over previous
"""Distributed MoE (top-2 routing, capacity 320) on 8 Trainium2 NeuronCores.

Sharding (matches the expert-parallel hint):
  - x is data-parallel sharded along B: core b owns batch row b (2048 tokens).
  - W1/b1/W2/b2 are sharded along the expert dim: core e owns expert e.
  - The router (Wg, bg) is replicated; each core routes its own tokens.
  - Dispatch: each core scatters its tokens into a [E, CAP, C] buffer and an
    AllToAll moves expert-e slabs to core e, which then holds [B, CAP, C]
    tokens for its expert. After the expert FFN a second AllToAll returns
    [E, CAP, C] outputs to each data-parallel core, which combines them with
    the gate probabilities.

Everything (router matmul, softmax, top-2, capacity positions via a prefix
scan, scatter/gather via indirect DMA, the two AllToAlls, and the expert FFN)
runs on-device; the host only slices/reassembles numpy arrays.

Key implementation points:
  - The dispatch buffer uses a chunk-major layout (row = j*G + e*CH + pos%CH,
    j = pos//CH) so each AllToAll is split into NG=5 chunked collectives whose
    transfers overlap the expert FFN groups (group g consumes chunk g).
  - Expert weights are passed host-pre-tiled as [out_chunk, 128, K*128] so one
    DMA per 128-wide output chunk loads all contraction tiles with contiguous
    16KB partition lines (the naive per-tile layout saturated the in-order
    sync sequencer and starved the PE).
  - FFN output DMAs ride the ACT HWDGE ring so they never block the sync ring
    that streams weights.
  - The token-position cumsum is a chained `tensor_tensor_scan`, fused per
    token tile so routing, index build, and dispatch scatter pipeline.
  - Matmuls default to fp16 (1 cycle/row vs 4 for fp32's two half-speed
    passes; measured rel. error 4.2e-4 vs the f32 reference, routing/top-k
    decisions are computed in exact f32 and match the reference bit-for-bit).
"""

import numpy as np

import concourse.mybir as mybir
import concourse.tile as tile
from concourse import bacc
from concourse.bass import IndirectOffsetOnAxis
from concourse.bass_utils import run_bass_kernel_spmd
from concourse.masks import make_identity

F32 = mybir.dt.float32
I32 = mybir.dt.int32
U32 = mybir.dt.uint32
AX = mybir.AxisListType
ALU = mybir.AluOpType
ACTF = mybir.ActivationFunctionType

P = 128


def build_moe_nc(T=2048, C=1024, E=8, CAP=320, DFF=4096, dt_mm1=F32, dt_mm2=F32, zero_disp=False):
    """Build the per-core (SPMD) Bass program. All 8 cores run this module."""
    assert T % P == 0 and C % P == 0 and DFF % P == 0
    NT = T // P         # token tiles per core
    KC = C // P         # C chunks (contraction for matmul1)
    KD = DFF // P       # DFF chunks (contraction for matmul2)
    ECAP = E * CAP      # rows in the dispatch buffer
    G = 512 if ECAP % 512 == 0 else ECAP   # FFN token-group size / A2A chunk rows
    assert ECAP % G == 0 and G % P == 0
    NG = ECAP // G      # FFN groups == A2A chunks
    NS = G // P         # 128-token subtiles per group
    CH = G // E         # capacity rows per (expert, chunk)
    SH = CH.bit_length() - 1
    assert (1 << SH) == CH, "chunk size must be a power of two"
    GSH = G.bit_length() - 1
    assert (1 << GSH) == G, "group size must be a power of two"
    assert CAP == NG * CH
    cores = list(range(E))

    nc = bacc.Bacc(None, target_bir_lowering=False, debug=False)

    # ---- I/O (per core) --------------------------------------------------
    x_ext = nc.dram_tensor("x", [T, C], F32, kind="ExternalInput")
    wg_ext = nc.dram_tensor("wgt", [P, KC, E], F32, kind="ExternalInput")   # Wg[C,E] -> [P, KC, E]
    bg_ext = nc.dram_tensor("bg", [1, E], F32, kind="ExternalInput")
    w1_ext = nc.dram_tensor("w1t", [KD, P, KC * P], dt_mm1, kind="ExternalInput")
    b1_ext = nc.dram_tensor("b1t", [P, KD], F32, kind="ExternalInput")
    w2_ext = nc.dram_tensor("w2t", [KC, P, KD * P], dt_mm2, kind="ExternalInput")
    b2_ext = nc.dram_tensor("b2t", [P, KC], F32, kind="ExternalInput")
    out_ext = nc.dram_tensor("out", [T, C], F32, kind="ExternalOutput")

    with tile.TileContext(nc) as tc:
        with (
            tc.tile_pool(name="const", bufs=1) as constp,
            tc.tile_pool(name="dram", bufs=1, space="DRAM") as dramp,
            tc.tile_pool(name="route", bufs=1) as routep,
        ):
            # ---- internal DRAM (collective + staging buffers) ----
            disp = dramp.tile([ECAP, C], F32)    # my tokens, per-expert slabs
            recv = dramp.tile([ECAP, C], F32)    # post-A2A: my expert, per-src slabs
            ysend = dramp.tile([ECAP, C], F32)   # expert outputs, per-src slabs
            recv2 = dramp.tile([ECAP, C], F32)   # post-A2A: my tokens' expert outputs

            # ---- constants ----
            ident = constp.tile([P, P], F32)
            make_identity(nc, ident)
            wg_sb = constp.tile([P, KC * E], F32)
            nc.sync.dma_start(wg_sb[:], wg_ext[:])
            bg_sb = constp.tile([1, E], F32)
            nc.sync.dma_start(bg_sb[:], bg_ext[:])
            ones1 = constp.tile([1, P], F32)
            nc.vector.memset(ones1[:], 1.0)
            ones8 = constp.tile([8, 1], F32)
            nc.vector.memset(ones8[:], 1.0)
            b1_sb = constp.tile([P, KD], F32)
            nc.sync.dma_start(b1_sb[:], b1_ext[:])
            b2_sb = constp.tile([P, KC], F32)
            nc.sync.dma_start(b2_sb[:], b2_ext[:])

            # ---- persistent routing tables (small; survive into combine) ----
            metas = [routep.tile([P, 8], F32, tag=f"meta{i}", name=f"meta{i}") for i in range(NT)]
            idxs = [routep.tile([P, 4], I32, tag=f"idx{i}", name=f"idx{i}") for i in range(NT)]

            # ================= Phase A: router + top-2 ====================
            with (
                tc.tile_pool(name="xa", bufs=1) as xap,
                tc.tile_pool(name="xtp", bufs=4) as xtp,
                tc.tile_pool(name="apsA", bufs=2, space="PSUM") as apsA,
                tc.tile_pool(name="apsB", bufs=2, space="PSUM") as apsB,
                tc.tile_pool(name="apsC", bufs=2, space="PSUM") as apsC,
                tc.tile_pool(name="apsD", bufs=1, space="PSUM") as apsD,
                tc.tile_pool(name="asb", bufs=4) as asb,
                tc.tile_pool(name="ascr", bufs=1) as ascr,
            ):
                # phase-A scratch (freed before the FFN needs the SBUF)
                SST = ascr.tile([8, T], F32)          # chained cumsum of expert one-hots
                if zero_disp:
                    # unfilled capacity slots never reach the output; zeroing
                    # only satisfies the simulator's NaN checker (emitted
                    # before the scatters, ordered via Tile WAW deps)
                    zt = asb.tile([P, C], F32, tag="zt", bufs=1)
                    nc.vector.memset(zt[:], 0.0)
                    for j in range(ECAP // P):
                        nc.gpsimd.dma_start(disp[j * P:(j + 1) * P, :], zt[:])
                x_tiles = []
                for i in range(NT):
                    x_t = xap.tile([P, C], F32, tag=f"x{i}", name=f"x{i}")
                    x_tiles.append(x_t)
                    nc.sync.dma_start(x_t[:], x_ext[i * P:(i + 1) * P, :])
                    # transpose x tile -> xT (C on partitions)
                    xT = xtp.tile([P, C], F32, tag="xT")
                    PK = min(4, KC)
                    for h in range(KC // PK):
                        xt_ps = apsA.tile([P, PK * P], F32, tag="xt_ps")
                        for q in range(PK):
                            k = h * PK + q
                            nc.tensor.transpose(
                                xt_ps[:, q * P:(q + 1) * P],
                                x_t[:, k * P:(k + 1) * P],
                                ident[:],
                            )
                        nc.scalar.copy(xT[:, h * PK * P:(h + 1) * PK * P], xt_ps[:])
                    # router logits: [P tokens, E]
                    lg_ps = apsB.tile([P, E], F32, tag="lg")
                    for k in range(KC):
                        nc.tensor.matmul(
                            lg_ps[:],
                            lhsT=xT[:, k * P:(k + 1) * P],
                            rhs=wg_sb[:, k * E:(k + 1) * E],
                            start=(k == 0),
                            stop=False,
                        )
                    nc.tensor.matmul(
                        lg_ps[:], lhsT=ones1[:], rhs=bg_sb[:], start=False, stop=True,
                    )
                    # softmax pieces (no normalization needed for top-k)
                    negm = asb.tile([P, 1], F32, tag="negm")
                    nc.vector.reduce_max(out=negm[:], in_=lg_ps[:], axis=AX.X, negate=True)
                    probs = asb.tile([P, E], F32, tag="probs")
                    nc.scalar.activation(probs[:], lg_ps[:], ACTF.Exp, bias=negm[:])
                    ssum = asb.tile([P, 1], F32, tag="ssum")
                    nc.vector.reduce_sum(out=ssum[:], in_=probs[:], axis=AX.X)
                    rinv = asb.tile([P, 1], F32, tag="rinv")
                    nc.vector.reciprocal(rinv[:], ssum[:])
                    mx8 = asb.tile([P, 8], F32, tag="mx8")
                    nc.vector.max(mx8[:], probs[:])
                    ix8 = asb.tile([P, 8], U32, tag="ix8")
                    nc.vector.max_index(ix8[:], mx8[:], probs[:])
                    # one-hots of the two selected experts, stacked [A | B]
                    ab = asb.tile([P, 16], F32, tag="ab")
                    nc.vector.tensor_scalar(
                        out=ab[:, 0:8], in0=probs[:], scalar1=mx8[:, 0:1],
                        scalar2=None, op0=ALU.is_equal,
                    )
                    nc.vector.tensor_scalar(
                        out=ab[:, 8:16], in0=probs[:], scalar1=mx8[:, 1:2],
                        scalar2=None, op0=ALU.is_equal,
                    )
                    meta = metas[i]
                    nc.vector.tensor_tensor(
                        out=meta[:, 0:1], in0=mx8[:, 0:1], in1=rinv[:], op=ALU.mult)
                    nc.vector.tensor_tensor(
                        out=meta[:, 1:2], in0=mx8[:, 1:2], in1=rinv[:], op=ALU.mult)
                    # transpose A and B -> [8, P] each
                    ab_ps = apsC.tile([8, 2 * P], F32, tag="ab_ps")
                    nc.tensor.transpose(ab_ps[:, 0:P], ab[:, 0:8], ident[:])
                    nc.tensor.transpose(ab_ps[:, P:2 * P], ab[:, 8:16], ident[:])
                    abt = asb.tile([8, 2 * P], F32, tag="abt")
                    nc.scalar.copy(abt[:], ab_ps[:])
                    # chained inclusive cumsum over tokens (per expert)
                    mt = asb.tile([8, P], F32, tag="mt")
                    nc.vector.tensor_tensor(
                        out=mt[:], in0=abt[:, 0:P], in1=abt[:, P:2 * P], op=ALU.add)
                    init = 0.0 if i == 0 else SST[:, i * P - 1:i * P]
                    nc.vector.tensor_tensor_scan(
                        out=SST[:, i * P:(i + 1) * P], data0=mt[:], data1=mt[:],
                        initial=init, op0=ALU.add, op1=ALU.bypass,
                    )
                    # extract this tile's inclusive positions for k=0 / k=1
                    prodt = asb.tile([8, 2 * P], F32, tag="prodt")
                    nc.vector.tensor_tensor(
                        out=prodt[:, 0:P], in0=abt[:, 0:P],
                        in1=SST[:, i * P:(i + 1) * P], op=ALU.mult)
                    nc.vector.tensor_tensor(
                        out=prodt[:, P:2 * P], in0=abt[:, P:2 * P],
                        in1=SST[:, i * P:(i + 1) * P], op=ALU.mult)
                    pos_ps = apsD.tile([1, 2 * P], F32, tag="pos_ps")
                    nc.tensor.matmul(
                        pos_ps[:, 0:P], lhsT=ones8[:], rhs=prodt[:, 0:P],
                        start=True, stop=True,
                    )
                    nc.tensor.matmul(
                        pos_ps[:, P:2 * P], lhsT=ones8[:], rhs=prodt[:, P:2 * P],
                        start=True, stop=True,
                    )
                    posr = asb.tile([1, 2 * P], F32, tag="posr")
                    nc.scalar.copy(posr[:], pos_ps[:])
                    pt_ps = apsD.tile([P, 2], F32, tag="pt_ps")
                    nc.tensor.transpose(pt_ps[:, 0:1], posr[:, 0:P], ident[0:1, 0:1])
                    nc.tensor.transpose(pt_ps[:, 1:2], posr[:, P:2 * P], ident[0:1, 0:1])
                    posT = asb.tile([P, 2], F32, tag="posT")
                    nc.vector.tensor_copy(posT[:], pt_ps[:])
                    keep = asb.tile([P, 2], F32, tag="keep")
                    nc.vector.tensor_scalar(
                        out=keep[:], in0=posT[:], scalar1=float(CAP),
                        scalar2=None, op0=ALU.is_le,
                    )
                    # gates = keep * topk_prob / sum
                    nc.vector.tensor_tensor(
                        out=meta[:, 4:5], in0=meta[:, 0:1], in1=keep[:, 0:1], op=ALU.mult)
                    nc.vector.tensor_tensor(
                        out=meta[:, 5:6], in0=meta[:, 1:2], in1=keep[:, 1:2], op=ALU.mult)
                    # dispatch row in chunk-major layout:
                    #   pos0 = pos_incl - 1, j = pos0 / CH (A2A chunk)
                    #   dst  = j*G + e*CH + pos0 % CH
                    pos_i = asb.tile([P, 2], I32, tag="pos_i")
                    nc.vector.tensor_copy(pos_i[:], posT[:])
                    nc.vector.tensor_scalar(
                        out=pos_i[:], in0=pos_i[:], scalar1=-1,
                        scalar2=None, op0=ALU.add)
                    e_i = asb.tile([P, 2], I32, tag="e_i")
                    nc.vector.tensor_copy(e_i[:, 0:1], ix8[:, 0:1])
                    nc.vector.tensor_copy(e_i[:, 1:2], ix8[:, 1:2])
                    jhi = asb.tile([P, 2], I32, tag="jhi")
                    nc.vector.tensor_scalar(
                        out=jhi[:], in0=pos_i[:], scalar1=SH, scalar2=GSH,
                        op0=ALU.arith_shift_right, op1=ALU.logical_shift_left)
                    dst_i = asb.tile([P, 2], I32, tag="dst_i")
                    nc.vector.tensor_scalar(
                        out=dst_i[:], in0=pos_i[:], scalar1=CH - 1,
                        scalar2=None, op0=ALU.bitwise_and)
                    nc.vector.tensor_tensor(
                        out=dst_i[:], in0=dst_i[:], in1=jhi[:], op=ALU.add)
                    esh = asb.tile([P, 2], I32, tag="esh")
                    nc.vector.tensor_scalar(
                        out=esh[:], in0=e_i[:], scalar1=SH,
                        scalar2=None, op0=ALU.logical_shift_left)
                    nc.vector.tensor_tensor(
                        out=dst_i[:], in0=dst_i[:], in1=esh[:], op=ALU.add)
                    keep_i = asb.tile([P, 2], I32, tag="keep_i")
                    nc.vector.tensor_copy(keep_i[:], keep[:])
                    idx = idxs[i]
                    nc.vector.memset(idx[:, 0:2], ECAP)       # dropped -> OOB, skipped
                    nc.vector.copy_predicated(idx[:, 0:2], keep_i[:], dst_i[:])
                    nc.vector.memset(idx[:, 2:4], 0)          # dropped -> row 0, gate 0
                    nc.vector.copy_predicated(idx[:, 2:4], keep_i[:], dst_i[:])
                    # dispatch scatter for this tile (both k-slots)
                    for k in range(2):
                        nc.gpsimd.indirect_dma_start(
                            out=disp[:, :],
                            out_offset=IndirectOffsetOnAxis(ap=idx[:, k:k + 1], axis=0),
                            in_=x_t[:, :],
                            in_offset=None,
                            bounds_check=ECAP - 1,
                            oob_is_err=False,
                        )
            # ================= Phase C: AllToAll (dispatch), chunked ======
            for j in range(NG):
                nc.gpsimd.collective_compute(
                    "AllToAll", ALU.bypass, replica_groups=[cores],
                    ins=[disp[j * G:(j + 1) * G, :].opt()],
                    outs=[recv[j * G:(j + 1) * G, :].opt()],
                )

            # ================= Phase D: expert FFN ========================
            half1 = dt_mm1 in (mybir.dt.float16, mybir.dt.bfloat16)
            half2 = dt_mm2 in (mybir.dt.float16, mybir.dt.bfloat16)
            with (
                tc.tile_pool(name="frecv", bufs=NS + 2 if half1 else NS + 1) as frecv,
                tc.tile_pool(name="fw1", bufs=8 if half1 else 4) as fw1,
                tc.tile_pool(name="fw2", bufs=3 if half2 else 2) as fw2,
                tc.tile_pool(name="ftokT", bufs=2) as ftokT,
                tc.tile_pool(name="fhT", bufs=2 if half2 else 1) as fhT,
                tc.tile_pool(name="fyT", bufs=2 if half2 else 1) as fyT,
                tc.tile_pool(name="fy", bufs=4) as fy,
                tc.tile_pool(name="fps_t", bufs=2, space="PSUM") as fps_t,
                tc.tile_pool(name="fps_h", bufs=2, space="PSUM") as fps_h,
                tc.tile_pool(name="fps_y", bufs=2, space="PSUM") as fps_y,
                tc.tile_pool(name="fps_o", bufs=2, space="PSUM") as fps_o,
            ):
                cast_tok = dt_mm1 in (mybir.dt.float16, mybir.dt.bfloat16)
                identh = None
                if cast_tok:
                    identh = constp.tile([P, P], dt_mm1, name="identh")
                    nc.vector.tensor_copy(identh[:], ident[:])
                for g in range(NG):
                    rts = []
                    for s in range(NS):
                        rt = frecv.tile([P, C], F32, tag="rt")
                        nc.scalar.dma_start(
                            rt[:], recv[(g * NS + s) * P:(g * NS + s + 1) * P, :])
                        if cast_tok:
                            # pre-cast on the idle DVE: the matmul would round
                            # to dt_mm1 anyway, and 16-bit PE transposes run 2x
                            rth = frecv.tile([P, C], dt_mm1, tag="rth")
                            nc.vector.tensor_copy(rth[:], rt[:])
                            rts.append(rth)
                        else:
                            rts.append(rt)
                    tokT = ftokT.tile([P, KC * G], dt_mm1, tag="tokT")
                    for k in range(KC):
                        tp = fps_t.tile([P, G], dt_mm1 if cast_tok else F32, tag="tp")
                        for s in range(NS):
                            nc.tensor.transpose(
                                tp[:, s * P:(s + 1) * P],
                                rts[s][:, k * P:(k + 1) * P],
                                identh[:] if cast_tok else ident[:],
                            )
                        nc.scalar.copy(tokT[:, k * G:(k + 1) * G], tp[:])
                    hT = fhT.tile([P, KD * G], dt_mm2, tag="hT")
                    for m in range(KD):
                        w1g = fw1.tile([P, KC * P], dt_mm1, tag="w1g")
                        nc.sync.dma_start(w1g[:], w1_ext[m])
                        hp = fps_h.tile([P, G], F32, tag="hp")
                        for k in range(KC):
                            nc.tensor.matmul(
                                hp[:], lhsT=w1g[:, k * P:(k + 1) * P],
                                rhs=tokT[:, k * G:(k + 1) * G],
                                start=(k == 0), stop=(k == KC - 1),
                            )
                        nc.scalar.activation(
                            hT[:, m * G:(m + 1) * G], hp[:], ACTF.Relu,
                            bias=b1_sb[:, m:m + 1],
                        )
                    yT = fyT.tile([P, KC * G], F32, tag="yT")
                    for mc in range(KC):
                        w2g = fw2.tile([P, KD * P], dt_mm2, tag="w2g")
                        nc.sync.dma_start(w2g[:], w2_ext[mc])
                        yp = fps_y.tile([P, G], F32, tag="yp")
                        for k in range(KD):
                            nc.tensor.matmul(
                                yp[:], lhsT=w2g[:, k * P:(k + 1) * P],
                                rhs=hT[:, k * G:(k + 1) * G],
                                start=(k == 0), stop=(k == KD - 1),
                            )
                        nc.scalar.activation(
                            yT[:, mc * G:(mc + 1) * G], yp[:], ACTF.Identity,
                            bias=b2_sb[:, mc:mc + 1],
                        )
                    # transpose back to [tokens, C] and store
                    PK = min(4, KC)
                    for s in range(NS):
                        y_t = fy.tile([P, C], F32, tag="y_t")
                        for h in range(KC // PK):
                            op_ps = fps_o.tile([P, PK * P], F32, tag="op_ps")
                            for q in range(PK):
                                mc = h * PK + q
                                nc.tensor.transpose(
                                    op_ps[:, q * P:(q + 1) * P],
                                    yT[:, mc * G + s * P: mc * G + (s + 1) * P],
                                    ident[:],
                                )
                            nc.scalar.copy(y_t[:, h * PK * P:(h + 1) * PK * P], op_ps[:])
                        nc.scalar.dma_start(
                            ysend[(g * NS + s) * P:(g * NS + s + 1) * P, :], y_t[:])

            # ================= Phase E: AllToAll (combine), chunked =======
            for j in range(NG):
                nc.gpsimd.collective_compute(
                    "AllToAll", ALU.bypass, replica_groups=[cores],
                    ins=[ysend[j * G:(j + 1) * G, :].opt()],
                    outs=[recv2[j * G:(j + 1) * G, :].opt()],
                )

            with (
                tc.tile_pool(name="cg", bufs=12) as cgp,
            ):
                for i in range(NT):
                    g0 = cgp.tile([P, C], F32, tag="g0")
                    nc.gpsimd.indirect_dma_start(
                        out=g0[:, :], out_offset=None,
                        in_=recv2[:, :],
                        in_offset=IndirectOffsetOnAxis(ap=idxs[i][:, 2:3], axis=0),
                    )
                    g1 = cgp.tile([P, C], F32, tag="g1")
                    nc.gpsimd.indirect_dma_start(
                        out=g1[:, :], out_offset=None,
                        in_=recv2[:, :],
                        in_offset=IndirectOffsetOnAxis(ap=idxs[i][:, 3:4], axis=0),
                    )
                    o_t = cgp.tile([P, C], F32, tag="o_t")
                    nc.scalar.activation(
                        o_t[:], g0[:], ACTF.Copy, scale=metas[i][:, 4:5])
                    g1s = cgp.tile([P, C], F32, tag="g1s")
                    nc.vector.tensor_scalar(
                        out=g1s[:], in0=g1[:], scalar1=metas[i][:, 5:6],
                        scalar2=None, op0=ALU.mult,
                    )
                    nc.vector.tensor_tensor(out=o_t[:], in0=o_t[:], in1=g1s[:], op=ALU.add)
                    nc.scalar.dma_start(out_ext[i * P:(i + 1) * P, :], o_t[:])

    nc.compile()
    return nc


# ---------------------------------------------------------------------------
# Host-side entry point
# ---------------------------------------------------------------------------

_NC_CACHE = {}


def _get_nc(key, **kw):
    if key not in _NC_CACHE:
        _NC_CACHE[key] = build_moe_nc(**kw)
    return _NC_CACHE[key]


def prep_inputs(x, Wg, bg, W1, b1, W2, b2, dt_np1=np.float32, dt_np2=np.float32):
    """Build the per-core input maps (host-side sharding / weight tiling)."""
    B, T, C = x.shape
    E, _, DFF = W1.shape
    KC, KD = C // P, DFF // P
    wgt = np.ascontiguousarray(
        np.asarray(Wg, np.float32).reshape(KC, P, E).transpose(1, 0, 2))
    bgr = np.asarray(bg, np.float32).reshape(1, E)
    in_maps = []
    for b in range(B):
        w1t = np.ascontiguousarray(
            np.asarray(W1[b], dt_np1).reshape(KC, P, KD, P).transpose(2, 1, 0, 3)
        ).reshape(KD, P, KC * P)
        w2t = np.ascontiguousarray(
            np.asarray(W2[b], dt_np2).reshape(KD, P, KC, P).transpose(2, 1, 0, 3)
        ).reshape(KC, P, KD * P)
        b1t = np.ascontiguousarray(np.asarray(b1[b], np.float32).reshape(KD, P).T)
        b2t = np.ascontiguousarray(np.asarray(b2[b], np.float32).reshape(KC, P).T)
        in_maps.append({
            "x": np.ascontiguousarray(np.asarray(x[b], np.float32)),
            "wgt": wgt, "bg": bgr,
            "w1t": w1t, "b1t": b1t, "w2t": w2t, "b2t": b2t,
        })
    return in_maps


def run_moe(x, Wg, bg, W1, b1, W2, b2, dt_mm1=F32, dt_mm2=F32, trace=False):
    B, T, C = x.shape
    E, _, DFF = W1.shape
    CAP = int(T / E * 1.25)
    nc = _get_nc((T, C, E, CAP, DFF, dt_mm1, dt_mm2),
                 T=T, C=C, E=E, CAP=CAP, DFF=DFF, dt_mm1=dt_mm1, dt_mm2=dt_mm2)

    def np_of(d):
        return np.float32 if d in (F32, mybir.dt.float32r) else mybir.dt.np(d)

    in_maps = prep_inputs(x, Wg, bg, W1, b1, W2, b2,
                          dt_np1=np_of(dt_mm1), dt_np2=np_of(dt_mm2))
    res = run_bass_kernel_spmd(nc, in_maps, list(range(E)), trace=trace)
    out = np.stack([res.results[b]["out"] for b in range(B)], axis=0)
    return out, res


DEFAULT_DT1 = mybir.dt.float16
DEFAULT_DT2 = mybir.dt.float16


def kernel(x, Wg, bg, W1, b1, W2, b2):
    out, _ = run_moe(
        np.asarray(x), np.asarray(Wg), np.asarray(bg), np.asarray(W1),
        np.asarray(b1), np.asarray(W2), np.asarray(b2),
        dt_mm1=DEFAULT_DT1, dt_mm2=DEFAULT_DT2,
    )
    return out
